# revision 11
# baseline (speedup 1.0000x reference)
"""Trainium2 Bass kernel for nn_CrossAttention (dense_transformer).

Reference computation (per batch b, per stream s in {1,2}):
    q_s   = heads(x_s)                      # [H, N, D] slices of x_s
    kv_s  = x_s @ Wkv_s -> k_s, v_s         # [N, C] each
    gate_s= sigmoid(relu(x_s @ w1 + b1) @ w2 + b2)
    ctx_s = softmax_d( scale * k_s^T @ (v_s * gate_s) )   # [H, D, D], softmax over d
    o_1   = q_1 @ ctx_2 ; o_2 = q_2 @ ctx_1  (cross)

Sharding: 8 cores = (stream s, batch b) pairs.  Core (s, b) projects
x_s[b] (kv + gate + ctx_s[b]) and then computes the OTHER stream's
output o_{1-s}[b] = q_{1-s}[b] @ softmax(ctx_s[b]).  No cross-core
communication; host concatenates outputs.
"""

import numpy as np
from contextlib import ExitStack

N = 4096
C = 1024
H = 16
D = 64
SCALE = D ** (-0.5)
NCH = N // 128       # 32 n-chunks of 128 rows
KCH = C // 128       # 8 contraction chunks
F32 = None           # set lazily (mybir import)

_CACHE = {}


def _build_program():
    """Build the SPMD Bass program (same for all 8 cores)."""
    import concourse.bass as bass
    import concourse.bacc as bacc
    import concourse.tile as tile
    import concourse.mybir as mybir

    F32 = mybir.dt.float32
    F32R = mybir.dt.float32r
    AF = mybir.ActivationFunctionType

    nc = bacc.Bacc("TRN2", target_bir_lowering=False, debug=False, num_devices=8)

    xp = nc.dram_tensor("xp", [N, C], F32, kind="ExternalInput").ap()
    xq = nc.dram_tensor("xq", [N, C], F32, kind="ExternalInput").ap()
    wkv = nc.dram_tensor("wkv", [C, 2 * C], F32, kind="ExternalInput").ap()
    w1 = nc.dram_tensor("w1", [C, C], F32, kind="ExternalInput").ap()
    b1 = nc.dram_tensor("b1", [C], F32, kind="ExternalInput").ap()
    w2 = nc.dram_tensor("w2", [C, C], F32, kind="ExternalInput").ap()
    b2 = nc.dram_tensor("b2", [C], F32, kind="ExternalInput").ap()
    ident = nc.dram_tensor("ident", [128, 128], F32, kind="ExternalInput").ap()
    o = nc.dram_tensor("o", [N, C], F32, kind="ExternalOutput").ap()


    with tile.TileContext(nc) as tc, ExitStack() as ctx:
        # ---------- persistent pools ----------
        cpool = ctx.enter_context(tc.tile_pool(name="consts", bufs=1))
        ident_sb = cpool.tile([128, 128], F32, name="ident_sb")
        nc.sync.dma_start(ident_sb, ident)
        ones_sb = cpool.tile([1, 128], F32, name="ones_sb")
        nc.vector.memset(ones_sb, 1.0)
        ones_r = cpool.tile([1, 128], F32R, name="ones_r")
        nc.vector.tensor_copy(ones_r, ones_sb)
        b1_sb = cpool.tile([128, 8], F32, name="b1_sb")  # b1_sb[p, m] = b1[m*128+p]
        nc.sync.dma_start(b1_sb, b1.rearrange("(m p) -> p m", p=128))
        b2_sb = cpool.tile([1, C], F32, name="b2_sb")
        nc.sync.dma_start(b2_sb, b2.rearrange("(one f) -> one f", one=1))
        b2_r = cpool.tile([1, C], F32R, name="b2_r")
        nc.vector.tensor_copy(b2_r, b2_sb)

        acc_pool = ctx.enter_context(tc.tile_pool(name="ctxacc", bufs=1))
        # ctxT accumulator on partitions 0-63: head h -> cols [h*64, h*64+64), layout [e, d]
        ctx_acc = acc_pool.tile([64, 1024], F32, name="ctx_acc")
        nc.vector.memset(ctx_acc, 0.0)

        spool = ctx.enter_context(tc.tile_pool(name="spairs", bufs=1))
        spairs = [spool.tile([128, 128], F32R, name=f"spair{j}") for j in range(8)]

        dpool = ctx.enter_context(tc.tile_pool(name="scratch", bufs=1, space="DRAM"))
        g_dram = dpool.tile([N, C], F32, name="g_dram")
        xpT_dram = dpool.tile([C, N], F32R, name="xpT_dram")

        # =========================================================
        # Phase A1: gate MLP for all n; also builds/spills xp^T.
        #   gate1 transposed-out: hT[m-tile, n] = (xp @ w1).T  (w1 stationary)
        #   gate2 normal-out:     g[n, :] = sigmoid(h @ w2 + b2)  (hT stationary)
        # =========================================================
        with ExitStack() as a1:
            wpool = a1.enter_context(tc.tile_pool(name="a1w", bufs=1))
            w1_sb = wpool.tile([128, 8, C], F32R, name="w1_sb")  # [p, k, col]
            w2_sb = wpool.tile([128, 8, C], F32R, name="w2_sb")
            for wsrc, wdst in ((w1, w1_sb), (w2, w2_sb)):
                for k in range(8):
                    wst = wpool.tile([128, C], F32, name="wst", tag="wst", bufs=2)
                    nc.sync.dma_start(
                        wst, wsrc[k * 128:(k + 1) * 128, :]
                    )
                    if k % 2 == 0:
                        nc.vector.tensor_copy(wdst[:, k, :], wst)
                    else:
                        nc.scalar.copy(wdst[:, k, :], wst)

            xin_pool = a1.enter_context(tc.tile_pool(name="a1xin", bufs=6))
            xpt_pool = a1.enter_context(tc.tile_pool(name="a1xpt", bufs=1))
            ht_pool = a1.enter_context(tc.tile_pool(name="a1ht", bufs=1))
            gout_pool = a1.enter_context(tc.tile_pool(name="a1g", bufs=2))
            trps_pool = a1.enter_context(
                tc.tile_pool(name="a1trps", bufs=2, space="PSUM")
            )
            g1ps_pool = a1.enter_context(
                tc.tile_pool(name="a1g1ps", bufs=4, space="PSUM")
            )
            g2ps_pool = a1.enter_context(
                tc.tile_pool(name="a1g2ps", bufs=2, space="PSUM")
            )

            for sb in range(4):  # superblocks of 1024 rows
                # xpT window tiles for this superblock: [128, 1024] per c-chunk j
                xpt = [
                    xpt_pool.tile([128, 1024], F32R, name=f"xpt{j}", tag=f"xpt{j}", bufs=1)
                    for j in range(8)
                ]
                for grp in range(2):  # 512-row halves
                    xins = []
                    for c4 in range(4):
                        xin = xin_pool.tile([128, C], F32, name="xin", tag="xin")
                        nch = sb * 8 + grp * 4 + c4
                        nc.sync.dma_start(xin, xp[nch * 128:(nch + 1) * 128, :])
                        xins.append(xin)
                    for j in range(8):
                        tps = trps_pool.tile([128, 512], F32, name="tps", tag="tps")
                        for c4 in range(4):
                            nc.tensor.transpose(
                                tps[:, c4 * 128:(c4 + 1) * 128],
                                xins[c4][:, j * 128:(j + 1) * 128],
                                ident_sb,
                            )
                        if j % 2 == 0:
                            nc.vector.tensor_copy(
                                xpt[j][:, grp * 512:(grp + 1) * 512], tps
                            )
                        else:
                            nc.scalar.copy(
                                xpt[j][:, grp * 512:(grp + 1) * 512], tps
                            )
                # spill xp^T
                for j in range(8):
                    nc.sync.dma_start(
                        xpT_dram[j * 128:(j + 1) * 128, sb * 1024:(sb + 1) * 1024],
                        xpt[j],
                    )
                # gate1 transposed: hT[m] = sum_k w1[k,m].T @ xpT[k]
                hts = [
                    ht_pool.tile([128, 1024], F32R, name=f"ht{m}", tag=f"ht{m}", bufs=1)
                    for m in range(8)
                ]
                for m in range(8):
                    pss = [
                        g1ps_pool.tile([128, 512], F32, name="g1ps", tag="g1ps")
                        for _ in range(2)
                    ]
                    for k in range(8):
                        lhs = w1_sb[:, k, m * 128:(m + 1) * 128]
                        for half in range(2):
                            nc.tensor.matmul(
                                pss[half],
                                lhs,
                                xpt[k][:, half * 512:(half + 1) * 512],
                                start=(k == 0),
                                stop=(k == 7),
                            )
                    for half in range(2):
                        nc.scalar.activation(
                            hts[m][:, half * 512:(half + 1) * 512],
                            pss[half],
                            AF.Relu,
                            bias=b1_sb[:, m:m + 1],
                        )
                # gate2 normal: g[n-chunk, :] = sigmoid(sum_k hT[k][:,nchunk].T @ w2[k] + b2)
                for c in range(8):
                    nch = sb * 8 + c
                    gt = gout_pool.tile([128, C], F32, name="gt", tag="gt")
                    for t in range(2):
                        ps2 = g2ps_pool.tile([128, 512], F32, name="g2ps", tag="g2ps")
                        for k in range(8):
                            nc.tensor.matmul(
                                ps2,
                                hts[k][:, c * 128:(c + 1) * 128],
                                w2_sb[:, k, t * 512:(t + 1) * 512],
                                start=(k == 0),
                                stop=False,
                            )
                        nc.tensor.matmul(
                            ps2,
                            ones_r,
                            b2_r[:, t * 512:(t + 1) * 512],
                            start=False,
                            stop=True,
                        )
                        nc.scalar.activation(
                            gt[:, t * 512:(t + 1) * 512], ps2, AF.Sigmoid
                        )
                    nc.sync.dma_start(g_dram[nch * 128:(nch + 1) * 128, :], gt)

        # =========================================================
        # Phase A2: kv projection + ctx accumulation.
        #   kv normal-out (xpT stationary); ctxT_h += vg_h.T @ k_h
        # =========================================================
        with ExitStack() as a2:
            wkv_pool = a2.enter_context(tc.tile_pool(name="a2w", bufs=1))
            wkv_sb = wkv_pool.tile([128, 8, 2 * C], F32R, name="wkv_sb")
            for k in range(8):
                wst2 = wkv_pool.tile([128, 2 * C], F32, name="wst2", tag="wst2", bufs=3)
                nc.sync.dma_start(wst2, wkv[k * 128:(k + 1) * 128, :])
                if k % 2 == 0:
                    nc.vector.tensor_copy(wkv_sb[:, k, :], wst2)
                else:
                    nc.scalar.copy(wkv_sb[:, k, :], wst2)

            xpt_in_pool = a2.enter_context(tc.tile_pool(name="a2xpt", bufs=3))
            gin_pool = a2.enter_context(tc.tile_pool(name="a2gin", bufs=3))
            k_pool = a2.enter_context(tc.tile_pool(name="a2k", bufs=2))
            v_pool = a2.enter_context(tc.tile_pool(name="a2v", bufs=2))
            vg_pool = a2.enter_context(tc.tile_pool(name="a2vg", bufs=2))
            kvps_pool = a2.enter_context(
                tc.tile_pool(name="a2kvps", bufs=6, space="PSUM")
            )
            ctps_pool = a2.enter_context(
                tc.tile_pool(name="a2ctps", bufs=1, space="PSUM")
            )

            for nch in range(NCH):
                xpt_in = xpt_in_pool.tile([128, C], F32R, name="xpt_in", tag="xpt_in")
                nc.sync.dma_start(
                    xpt_in,
                    xpT_dram.rearrange("(k p) n -> p k n", p=128)[
                        :, :, nch * 128:(nch + 1) * 128
                    ],
                )
                gin = gin_pool.tile([128, C], F32, name="gin", tag="gin")
                nc.sync.dma_start(gin, g_dram[nch * 128:(nch + 1) * 128, :])

                kvps = [
                    kvps_pool.tile([128, 512], F32, name="kvps", tag="kvps")
                    for _ in range(4)
                ]
                for k in range(8):
                    lhs = xpt_in[:, k * 128:(k + 1) * 128]
                    for t in range(4):
                        nc.tensor.matmul(
                            kvps[t],
                            lhs,
                            wkv_sb[:, k, t * 512:(t + 1) * 512],
                            start=(k == 0),
                            stop=(k == 7),
                        )
                k_sb = k_pool.tile([128, C], F32R, name="k_sb", tag="k_sb")
                v_sb = v_pool.tile([128, C], F32, name="v_sb", tag="v_sb")
                nc.scalar.copy(k_sb[:, 0:512], kvps[0])
                nc.scalar.copy(k_sb[:, 512:1024], kvps[1])
                nc.vector.tensor_copy(v_sb[:, 0:512], kvps[2])
                nc.vector.tensor_copy(v_sb[:, 512:1024], kvps[3])
                vg = vg_pool.tile([128, C], F32R, name="vg", tag="vg")
                nc.vector.tensor_mul(vg, v_sb, gin)

                ctp = ctps_pool.tile([64, 1024], F32, name="ctp", tag="ctp")
                for h in range(H):
                    nc.tensor.matmul(
                        ctp[:, h * D:(h + 1) * D],
                        vg[:, h * D:(h + 1) * D],
                        k_sb[:, h * D:(h + 1) * D],
                        start=True,
                        stop=True,
                        skip_group_check=True,
                    )
                nc.vector.tensor_add(ctx_acc, ctx_acc, ctp)

        # =========================================================
        # Softmax over d (free dim of ctxT) + build block-diag S pairs
        # =========================================================
        with ExitStack() as sm:
            smp = sm.enter_context(tc.tile_pool(name="smpool", bufs=1))
            smps = sm.enter_context(tc.tile_pool(name="smps", bufs=2, space="PSUM"))
            maxs = smp.tile([64, 16], F32, name="maxs")
            nc.vector.tensor_reduce(
                maxs,
                ctx_acc.rearrange("p (b d) -> p b d", b=16),
                axis=mybir.AxisListType.X,
                op=mybir.AluOpType.max,
            )
            ebias = smp.tile([64, 16], F32, name="ebias")
            nc.vector.tensor_scalar_mul(ebias, maxs, -SCALE)
            et = smp.tile([64, 1024], F32, name="et")
            for h in range(16):
                nc.scalar.activation(
                    et[:, h * 64:(h + 1) * 64],
                    ctx_acc[:, h * 64:(h + 1) * 64],
                    AF.Exp,
                    bias=ebias[:, h:h + 1],
                    scale=float(SCALE),
                )
            sums = smp.tile([64, 16], F32, name="sums")
            nc.vector.tensor_reduce(
                sums,
                et.rearrange("p (b d) -> p b d", b=16),
                axis=mybir.AxisListType.X,
                op=mybir.AluOpType.add,
            )
            recs = smp.tile([64, 16], F32, name="recs")
            nc.vector.reciprocal(recs, sums)
            st = smp.tile([64, 1024], F32, name="st")
            for h in range(16):
                nc.vector.tensor_scalar_mul(
                    st[:, h * 64:(h + 1) * 64],
                    et[:, h * 64:(h + 1) * 64],
                    recs[:, h:h + 1],
                )
            # st: softmaxed ctxT [e, d] per head at cols h*64.  Transposing the
            # side-by-side pair [ctxT_2j | ctxT_2j+1] ([64, 128]) gives
            # [S_2j stacked above S_2j+1] ([128, 64]); scatter to block-diag.
            zero_sb = smp.tile([128, 128], F32, name="zero_sb")
            nc.vector.memset(zero_sb, 0.0)
            for j in range(8):
                tp = smps.tile([128, 64], F32, name="smtp", tag="smtp")
                nc.tensor.transpose(
                    tp, st[:, (2 * j) * 64:(2 * j + 2) * 64], ident_sb[0:64, 0:64]
                )
                nc.vector.tensor_copy(spairs[j], zero_sb)
                nc.vector.tensor_copy(spairs[j][0:64, 0:64], tp[0:64, :])
                nc.vector.tensor_copy(spairs[j][64:128, 64:128], tp[64:128, :])

        # =========================================================
        # Phase B: o = q @ S.  oT[j] = spair_j.T @ xqT[j], then
        # transpose back to normal layout and DMA out.
        # =========================================================
        with ExitStack() as pb:
            xin_pool = pb.enter_context(tc.tile_pool(name="bxin", bufs=6))
            xqt_pool = pb.enter_context(tc.tile_pool(name="bxqt", bufs=1))
            ot_pool = pb.enter_context(tc.tile_pool(name="bot", bufs=1))
            oout_pool = pb.enter_context(tc.tile_pool(name="bo", bufs=6))
            trps_pool = pb.enter_context(tc.tile_pool(name="btrps", bufs=2, space="PSUM"))
            otps_pool = pb.enter_context(tc.tile_pool(name="botps", bufs=2, space="PSUM"))
            btps_pool = pb.enter_context(tc.tile_pool(name="bbtps", bufs=2, space="PSUM"))

            for blk in range(8):  # 512-row blocks
                xins = []
                for c4 in range(4):
                    xin = xin_pool.tile([128, C], F32, name="bxin", tag="bxin")
                    nch = blk * 4 + c4
                    nc.sync.dma_start(xin, xq[nch * 128:(nch + 1) * 128, :])
                    xins.append(xin)
                xqts = [
                    xqt_pool.tile([128, 512], F32R, name=f"xqt{j}", tag=f"xqt{j}", bufs=2)
                    for j in range(8)
                ]
                for j in range(8):
                    tps = trps_pool.tile([128, 512], F32, name="btps", tag="btps")
                    for c4 in range(4):
                        nc.tensor.transpose(
                            tps[:, c4 * 128:(c4 + 1) * 128],
                            xins[c4][:, j * 128:(j + 1) * 128],
                            ident_sb,
                        )
                    if j % 2 == 0:
                        nc.vector.tensor_copy(xqts[j], tps)
                    else:
                        nc.scalar.copy(xqts[j], tps)
                ots = [
                    ot_pool.tile([128, 512], F32, name=f"ot{j}", tag=f"ot{j}", bufs=2)
                    for j in range(8)
                ]
                for j in range(8):
                    ops = otps_pool.tile([128, 512], F32, name="ops", tag="ops")
                    nc.tensor.matmul(
                        ops, spairs[j], xqts[j], start=True, stop=True
                    )
                    nc.scalar.copy(ots[j], ops)
                oouts = [
                    oout_pool.tile([128, C], F32, name="oo", tag="oo")
                    for _ in range(4)
                ]
                for half in range(2):
                    for c4 in range(4):
                        bps = btps_pool.tile([128, 512], F32, name="bps", tag="bps")
                        for jj in range(4):
                            j = half * 4 + jj
                            nc.tensor.transpose(
                                bps[:, jj * 128:(jj + 1) * 128],
                                ots[j][:, c4 * 128:(c4 + 1) * 128],
                                ident_sb,
                            )
                        if c4 % 2 == 0:
                            nc.vector.tensor_copy(
                                oouts[c4][:, half * 512:(half + 1) * 512], bps
                            )
                        else:
                            nc.scalar.copy(
                                oouts[c4][:, half * 512:(half + 1) * 512], bps
                            )
                for c4 in range(4):
                    nch = blk * 4 + c4
                    nc.sync.dma_start(o[nch * 128:(nch + 1) * 128, :], oouts[c4])

    nc.compile()
    return nc


def _get_program():
    if "nc" not in _CACHE:
        _CACHE["nc"] = _build_program()
    return _CACHE["nc"]


def make_in_maps(x1, x2, Wkv1, Wkv2, g1_w1, g1_b1, g1_w2, g1_b2,
                 g2_w1, g2_b1, g2_w2, g2_b2):
    """Core (s, b): cores 0-3 = (s=0, b), cores 4-7 = (s=1, b)."""
    ident = np.eye(128, dtype=np.float32)
    asf = np.ascontiguousarray
    in_maps = []
    for core in range(8):
        s, b = core // 4, core % 4
        if s == 0:
            m = dict(xp=asf(x1[b]), xq=asf(x2[b]), wkv=asf(Wkv1),
                     w1=asf(g1_w1), b1=asf(g1_b1), w2=asf(g1_w2), b2=asf(g1_b2))
        else:
            m = dict(xp=asf(x2[b]), xq=asf(x1[b]), wkv=asf(Wkv2),
                     w1=asf(g2_w1), b1=asf(g2_b1), w2=asf(g2_w2), b2=asf(g2_b2))
        m["ident"] = ident
        in_maps.append(m)
    return in_maps


def kernel(x1, x2, Wkv1, Wkv2, g1_w1, g1_b1, g1_w2, g1_b2,
           g2_w1, g2_b1, g2_w2, g2_b2, _runner=None):
    """Full-input entry point.  Returns (o1, o2), each [4, 4096, 1024] f32."""
    from concourse.bass_utils import run_bass_kernel_spmd

    args = [np.asarray(a, dtype=np.float32) for a in
            (x1, x2, Wkv1, Wkv2, g1_w1, g1_b1, g1_w2, g1_b2,
             g2_w1, g2_b1, g2_w2, g2_b2)]
    nc = _get_program()
    in_maps = make_in_maps(*args)
    if _runner is None:
        res = run_bass_kernel_spmd(nc, in_maps, core_ids=list(range(8)))
        results = res.results
    else:
        results = _runner(nc, in_maps)

    B = x1.shape[0]
    o1 = np.empty((B, N, C), dtype=np.float32)
    o2 = np.empty((B, N, C), dtype=np.float32)
    for core in range(8):
        s, b = core // 4, core % 4
        out = results[core]["o"]
        if s == 0:
            o2[b] = out   # core projected x1 -> ctx1 -> o2 = q2 @ ctx1
        else:
            o1[b] = out
    return (o1, o2)


# revision 12
# speedup vs baseline: 1.0556x; 1.0556x over previous
"""Trainium2 Bass kernel for nn_CrossAttention (dense_transformer).

Reference computation (per batch b, per stream s in {1,2}):
    q_s   = heads(x_s)                      # [H, N, D] slices of x_s
    kv_s  = x_s @ Wkv_s -> k_s, v_s         # [N, C] each
    gate_s= sigmoid(relu(x_s @ w1 + b1) @ w2 + b2)
    ctx_s = softmax_d( scale * k_s^T @ (v_s * gate_s) )   # [H, D, D], softmax over d
    o_1   = q_1 @ ctx_2 ; o_2 = q_2 @ ctx_1  (cross)

Sharding: 8 cores = (stream s, batch b) pairs.  Core (s, b) projects
x_s[b] (kv + gate + ctx_s[b]) and then computes the OTHER stream's
output o_{1-s}[b] = q_{1-s}[b] @ softmax(ctx_s[b]).  No cross-core
communication; host concatenates outputs.
"""

import numpy as np
from contextlib import ExitStack

N = 4096
C = 1024
H = 16
D = 64
SCALE = D ** (-0.5)
NCH = N // 128       # 32 n-chunks of 128 rows
KCH = C // 128       # 8 contraction chunks
F32 = None           # set lazily (mybir import)

_CACHE = {}


def _build_program(with_bias):
    """Build the SPMD Bass program (same for all 8 cores)."""
    import concourse.bass as bass
    import concourse.bacc as bacc
    import concourse.tile as tile
    import concourse.mybir as mybir

    F32 = mybir.dt.float32
    F32R = mybir.dt.float32r
    AF = mybir.ActivationFunctionType

    nc = bacc.Bacc("TRN2", target_bir_lowering=False, debug=False, num_devices=8)

    xp = nc.dram_tensor("xp", [N, C], F32R, kind="ExternalInput").ap()
    xq = nc.dram_tensor("xq", [N, C], F32R, kind="ExternalInput").ap()
    wkv = nc.dram_tensor("wkv", [C, 2 * C], F32R, kind="ExternalInput").ap()
    w1 = nc.dram_tensor("w1", [C, C], F32R, kind="ExternalInput").ap()
    b1 = nc.dram_tensor("b1", [C], F32, kind="ExternalInput").ap()
    w2 = nc.dram_tensor("w2", [C, C], F32R, kind="ExternalInput").ap()
    b2 = nc.dram_tensor("b2", [C], F32R, kind="ExternalInput").ap()
    ident = nc.dram_tensor("ident", [128, 128], F32R, kind="ExternalInput").ap()
    o = nc.dram_tensor("o", [N, C], F32, kind="ExternalOutput").ap()


    with tile.TileContext(nc) as tc, ExitStack() as ctx:
        # ---------- persistent pools ----------
        cpool = ctx.enter_context(tc.tile_pool(name="consts", bufs=1))
        ident_sb = cpool.tile([128, 128], F32R, name="ident_sb")
        nc.sync.dma_start(ident_sb, ident)
        identf = cpool.tile([128, 128], F32, name="identf")
        nc.vector.tensor_copy(identf, ident_sb)
        b1_sb = cpool.tile([128, 8], F32, name="b1_sb")  # b1_sb[p, m] = b1[m*128+p]
        nc.sync.dma_start(b1_sb, b1.rearrange("(m p) -> p m", p=128))
        if with_bias:
            ones_sb = cpool.tile([1, 128], F32, name="ones_sb")
            nc.vector.memset(ones_sb, 1.0)
            ones_r = cpool.tile([1, 128], F32R, name="ones_r")
            nc.vector.tensor_copy(ones_r, ones_sb)
            b2_r = cpool.tile([1, C], F32R, name="b2_r")
            nc.sync.dma_start(b2_r, b2.rearrange("(one f) -> one f", one=1))

        acc_pool = ctx.enter_context(tc.tile_pool(name="ctxacc", bufs=1))
        # ctxT accumulator on partitions 0-63: head h -> cols [h*64, h*64+64), layout [e, d]
        ctx_acc = acc_pool.tile([64, 1024], F32, name="ctx_acc")
        nc.vector.memset(ctx_acc, 0.0)

        spool = ctx.enter_context(tc.tile_pool(name="spairs", bufs=1))
        spairs = [spool.tile([128, 128], F32R, name=f"spair{j}") for j in range(8)]

        dpool = ctx.enter_context(tc.tile_pool(name="scratch", bufs=1, space="DRAM"))
        g_dram = dpool.tile([N, C], F32, name="g_dram")
        xpT_dram = dpool.tile([C, N], F32R, name="xpT_dram")

        # =========================================================
        # Phase A1: gate MLP for all n; also builds/spills xp^T.
        #   gate1 transposed-out: hT[m-tile, n] = (xp @ w1).T  (w1 stationary)
        #   gate2 normal-out:     g[n, :] = sigmoid(h @ w2 + b2)  (hT stationary)
        # =========================================================
        with ExitStack() as a1:
            wpool = a1.enter_context(tc.tile_pool(name="a1w", bufs=1))
            w1_sb = wpool.tile([128, 8, C], F32R, name="w1_sb")  # [p, k, col]
            nc.sync.dma_start(w1_sb, w1.rearrange("(k p) m -> p k m", p=128))
            w2_sb = wpool.tile([128, 8, C], F32R, name="w2_sb")
            nc.sync.dma_start(w2_sb, w2.rearrange("(k p) m -> p k m", p=128))

            xin_pool = a1.enter_context(tc.tile_pool(name="a1xin", bufs=6))
            xpt_pool = a1.enter_context(tc.tile_pool(name="a1xpt", bufs=1))
            ht_pool = a1.enter_context(tc.tile_pool(name="a1ht", bufs=1))
            gout_pool = a1.enter_context(tc.tile_pool(name="a1g", bufs=2))
            trps_pool = a1.enter_context(
                tc.tile_pool(name="a1trps", bufs=2, space="PSUM")
            )
            g1ps_pool = a1.enter_context(
                tc.tile_pool(name="a1g1ps", bufs=4, space="PSUM")
            )
            g2ps_pool = a1.enter_context(
                tc.tile_pool(name="a1g2ps", bufs=2, space="PSUM")
            )

            for sb in range(4):  # superblocks of 1024 rows
                # xpT window tiles for this superblock: [128, 1024] per c-chunk j
                xpt = [
                    xpt_pool.tile([128, 1024], F32R, name=f"xpt{j}", tag=f"xpt{j}", bufs=1)
                    for j in range(8)
                ]
                for grp in range(2):  # 512-row halves
                    xins = []
                    for c4 in range(4):
                        xin = xin_pool.tile([128, C], F32R, name="xin", tag="xin")
                        nch = sb * 8 + grp * 4 + c4
                        nc.sync.dma_start(xin, xp[nch * 128:(nch + 1) * 128, :])
                        xins.append(xin)
                    for j in range(8):
                        tps = trps_pool.tile([128, 512], F32R, name="tps", tag="tps")
                        for c4 in range(4):
                            nc.tensor.transpose(
                                tps[:, c4 * 128:(c4 + 1) * 128],
                                xins[c4][:, j * 128:(j + 1) * 128],
                                ident_sb,
                            )
                        if j % 2 == 0:
                            nc.vector.tensor_copy(
                                xpt[j][:, grp * 512:(grp + 1) * 512], tps
                            )
                        else:
                            nc.scalar.copy(
                                xpt[j][:, grp * 512:(grp + 1) * 512], tps
                            )
                # spill xp^T
                for j in range(8):
                    nc.sync.dma_start(
                        xpT_dram[j * 128:(j + 1) * 128, sb * 1024:(sb + 1) * 1024],
                        xpt[j],
                    )
                # gate1 transposed: hT[m] = sum_k w1[k,m].T @ xpT[k]
                hts = [
                    ht_pool.tile([128, 1024], F32R, name=f"ht{m}", tag=f"ht{m}", bufs=1)
                    for m in range(8)
                ]
                for m in range(8):
                    pss = [
                        g1ps_pool.tile([128, 512], F32, name="g1ps", tag="g1ps")
                        for _ in range(2)
                    ]
                    for k in range(8):
                        lhs = w1_sb[:, k, m * 128:(m + 1) * 128]
                        for half in range(2):
                            nc.tensor.matmul(
                                pss[half],
                                lhs,
                                xpt[k][:, half * 512:(half + 1) * 512],
                                start=(k == 0),
                                stop=(k == 7),
                            )
                    for half in range(2):
                        nc.scalar.activation(
                            hts[m][:, half * 512:(half + 1) * 512],
                            pss[half],
                            AF.Relu,
                            bias=b1_sb[:, m:m + 1],
                        )
                # gate2 normal: g[n-chunk, :] = sigmoid(sum_k hT[k][:,nchunk].T @ w2[k] + b2)
                for c in range(8):
                    nch = sb * 8 + c
                    gt = gout_pool.tile([128, C], F32, name="gt", tag="gt")
                    for t in range(2):
                        ps2 = g2ps_pool.tile([128, 512], F32, name="g2ps", tag="g2ps")
                        for k in range(8):
                            nc.tensor.matmul(
                                ps2,
                                hts[k][:, c * 128:(c + 1) * 128],
                                w2_sb[:, k, t * 512:(t + 1) * 512],
                                start=(k == 0),
                                stop=(k == 7 and not with_bias),
                            )
                        if with_bias:
                            nc.tensor.matmul(
                                ps2,
                                ones_r,
                                b2_r[:, t * 512:(t + 1) * 512],
                                start=False,
                                stop=True,
                            )
                        nc.scalar.activation(
                            gt[:, t * 512:(t + 1) * 512], ps2, AF.Sigmoid
                        )
                    nc.sync.dma_start(g_dram[nch * 128:(nch + 1) * 128, :], gt)

        # =========================================================
        # Phase A2: kv projection + ctx accumulation.
        #   kv normal-out (xpT stationary); ctxT_h += vg_h.T @ k_h
        # =========================================================
        with ExitStack() as a2:
            wkv_pool = a2.enter_context(tc.tile_pool(name="a2w", bufs=1))
            wkv_sb = wkv_pool.tile([128, 8, 2 * C], F32R, name="wkv_sb")
            nc.sync.dma_start(wkv_sb, wkv.rearrange("(k p) m -> p k m", p=128))

            xpt_in_pool = a2.enter_context(tc.tile_pool(name="a2xpt", bufs=3))
            gin_pool = a2.enter_context(tc.tile_pool(name="a2gin", bufs=3))
            k_pool = a2.enter_context(tc.tile_pool(name="a2k", bufs=2))
            v_pool = a2.enter_context(tc.tile_pool(name="a2v", bufs=2))
            vg_pool = a2.enter_context(tc.tile_pool(name="a2vg", bufs=2))
            kvps_pool = a2.enter_context(
                tc.tile_pool(name="a2kvps", bufs=6, space="PSUM")
            )
            ctps_pool = a2.enter_context(
                tc.tile_pool(name="a2ctps", bufs=1, space="PSUM")
            )

            for nch in range(NCH):
                xpt_in = xpt_in_pool.tile([128, C], F32R, name="xpt_in", tag="xpt_in")
                nc.sync.dma_start(
                    xpt_in,
                    xpT_dram.rearrange("(k p) n -> p k n", p=128)[
                        :, :, nch * 128:(nch + 1) * 128
                    ],
                )
                gin = gin_pool.tile([128, C], F32, name="gin", tag="gin")
                nc.sync.dma_start(gin, g_dram[nch * 128:(nch + 1) * 128, :])

                kvps = [
                    kvps_pool.tile([128, 512], F32, name="kvps", tag="kvps")
                    for _ in range(4)
                ]
                for k in range(8):
                    lhs = xpt_in[:, k * 128:(k + 1) * 128]
                    for t in range(4):
                        nc.tensor.matmul(
                            kvps[t],
                            lhs,
                            wkv_sb[:, k, t * 512:(t + 1) * 512],
                            start=(k == 0),
                            stop=(k == 7),
                        )
                k_sb = k_pool.tile([128, C], F32R, name="k_sb", tag="k_sb")
                v_sb = v_pool.tile([128, C], F32, name="v_sb", tag="v_sb")
                nc.scalar.copy(k_sb[:, 0:512], kvps[0])
                nc.scalar.copy(k_sb[:, 512:1024], kvps[1])
                nc.vector.tensor_copy(v_sb[:, 0:512], kvps[2])
                nc.vector.tensor_copy(v_sb[:, 512:1024], kvps[3])
                vg = vg_pool.tile([128, C], F32R, name="vg", tag="vg")
                nc.vector.tensor_mul(vg, v_sb, gin)

                ctp = ctps_pool.tile([64, 1024], F32, name="ctp", tag="ctp")
                for h in range(H):
                    nc.tensor.matmul(
                        ctp[:, h * D:(h + 1) * D],
                        vg[:, h * D:(h + 1) * D],
                        k_sb[:, h * D:(h + 1) * D],
                        start=True,
                        stop=True,
                        skip_group_check=True,
                    )
                nc.vector.tensor_add(ctx_acc, ctx_acc, ctp)

        # =========================================================
        # Softmax over d (free dim of ctxT) + build block-diag S pairs
        # =========================================================
        with ExitStack() as sm:
            smp = sm.enter_context(tc.tile_pool(name="smpool", bufs=1))
            smps = sm.enter_context(tc.tile_pool(name="smps", bufs=2, space="PSUM"))
            maxs = smp.tile([64, 16], F32, name="maxs")
            nc.vector.tensor_reduce(
                maxs,
                ctx_acc.rearrange("p (b d) -> p b d", b=16),
                axis=mybir.AxisListType.X,
                op=mybir.AluOpType.max,
            )
            ebias = smp.tile([64, 16], F32, name="ebias")
            nc.vector.tensor_scalar_mul(ebias, maxs, -SCALE)
            et = smp.tile([64, 1024], F32, name="et")
            for h in range(16):
                nc.scalar.activation(
                    et[:, h * 64:(h + 1) * 64],
                    ctx_acc[:, h * 64:(h + 1) * 64],
                    AF.Exp,
                    bias=ebias[:, h:h + 1],
                    scale=float(SCALE),
                )
            sums = smp.tile([64, 16], F32, name="sums")
            nc.vector.tensor_reduce(
                sums,
                et.rearrange("p (b d) -> p b d", b=16),
                axis=mybir.AxisListType.X,
                op=mybir.AluOpType.add,
            )
            recs = smp.tile([64, 16], F32, name="recs")
            nc.vector.reciprocal(recs, sums)
            st = smp.tile([64, 1024], F32, name="st")
            for h in range(16):
                nc.vector.tensor_scalar_mul(
                    st[:, h * 64:(h + 1) * 64],
                    et[:, h * 64:(h + 1) * 64],
                    recs[:, h:h + 1],
                )
            # st: softmaxed ctxT [e, d] per head at cols h*64.  Transposing the
            # side-by-side pair [ctxT_2j | ctxT_2j+1] ([64, 128]) gives
            # [S_2j stacked above S_2j+1] ([128, 64]); scatter to block-diag.
            zero_sb = smp.tile([128, 128], F32, name="zero_sb")
            nc.vector.memset(zero_sb, 0.0)
            for j in range(8):
                tp = smps.tile([128, 64], F32, name="smtp", tag="smtp")
                nc.tensor.transpose(
                    tp, st[:, (2 * j) * 64:(2 * j + 2) * 64], identf[0:64, 0:64]
                )
                nc.vector.tensor_copy(spairs[j], zero_sb)
                nc.vector.tensor_copy(spairs[j][0:64, 0:64], tp[0:64, :])
                nc.vector.tensor_copy(spairs[j][64:128, 64:128], tp[64:128, :])

        # =========================================================
        # Phase B: o = q @ S.  oT[j] = spair_j.T @ xqT[j], then
        # transpose back to normal layout and DMA out.
        # =========================================================
        with ExitStack() as pb:
            xin_pool = pb.enter_context(tc.tile_pool(name="bxin", bufs=6))
            xqt_pool = pb.enter_context(tc.tile_pool(name="bxqt", bufs=1))
            ot_pool = pb.enter_context(tc.tile_pool(name="bot", bufs=1))
            oout_pool = pb.enter_context(tc.tile_pool(name="bo", bufs=6))
            trps_pool = pb.enter_context(tc.tile_pool(name="btrps", bufs=2, space="PSUM"))
            otps_pool = pb.enter_context(tc.tile_pool(name="botps", bufs=2, space="PSUM"))
            btps_pool = pb.enter_context(tc.tile_pool(name="bbtps", bufs=2, space="PSUM"))

            for blk in range(8):  # 512-row blocks
                xins = []
                for c4 in range(4):
                    xin = xin_pool.tile([128, C], F32R, name="bxin", tag="bxin")
                    nch = blk * 4 + c4
                    nc.sync.dma_start(xin, xq[nch * 128:(nch + 1) * 128, :])
                    xins.append(xin)
                xqts = [
                    xqt_pool.tile([128, 512], F32R, name=f"xqt{j}", tag=f"xqt{j}", bufs=2)
                    for j in range(8)
                ]
                for j in range(8):
                    tps = trps_pool.tile([128, 512], F32R, name="btps", tag="btps")
                    for c4 in range(4):
                        nc.tensor.transpose(
                            tps[:, c4 * 128:(c4 + 1) * 128],
                            xins[c4][:, j * 128:(j + 1) * 128],
                            ident_sb,
                        )
                    if j % 2 == 0:
                        nc.vector.tensor_copy(xqts[j], tps)
                    else:
                        nc.scalar.copy(xqts[j], tps)
                ots = [
                    ot_pool.tile([128, 512], F32, name=f"ot{j}", tag=f"ot{j}", bufs=2)
                    for j in range(8)
                ]
                for j in range(8):
                    ops = otps_pool.tile([128, 512], F32, name="ops", tag="ops")
                    nc.tensor.matmul(
                        ops, spairs[j], xqts[j], start=True, stop=True
                    )
                    nc.scalar.copy(ots[j], ops)
                oouts = [
                    oout_pool.tile([128, C], F32, name="oo", tag="oo")
                    for _ in range(4)
                ]
                for half in range(2):
                    for c4 in range(4):
                        bps = btps_pool.tile([128, 512], F32, name="bps", tag="bps")
                        for jj in range(4):
                            j = half * 4 + jj
                            nc.tensor.transpose(
                                bps[:, jj * 128:(jj + 1) * 128],
                                ots[j][:, c4 * 128:(c4 + 1) * 128],
                                identf,
                            )
                        if c4 % 2 == 0:
                            nc.vector.tensor_copy(
                                oouts[c4][:, half * 512:(half + 1) * 512], bps
                            )
                        else:
                            nc.scalar.copy(
                                oouts[c4][:, half * 512:(half + 1) * 512], bps
                            )
                for c4 in range(4):
                    nch = blk * 4 + c4
                    nc.sync.dma_start(o[nch * 128:(nch + 1) * 128, :], oouts[c4])

    nc.compile()
    return nc


def _get_program(with_bias=False):
    key = ("nc", bool(with_bias))
    if key not in _CACHE:
        _CACHE[key] = _build_program(with_bias)
    return _CACHE[key]


def make_in_maps(x1, x2, Wkv1, Wkv2, g1_w1, g1_b1, g1_w2, g1_b2,
                 g2_w1, g2_b1, g2_w2, g2_b2):
    """Core (s, b): cores 0-3 = (s=0, b), cores 4-7 = (s=1, b)."""
    ident = np.eye(128, dtype=np.float32)
    asf = np.ascontiguousarray
    in_maps = []
    for core in range(8):
        s, b = core // 4, core % 4
        if s == 0:
            m = dict(xp=asf(x1[b]), xq=asf(x2[b]), wkv=asf(Wkv1),
                     w1=asf(g1_w1), b1=asf(g1_b1), w2=asf(g1_w2), b2=asf(g1_b2))
        else:
            m = dict(xp=asf(x2[b]), xq=asf(x1[b]), wkv=asf(Wkv2),
                     w1=asf(g2_w1), b1=asf(g2_b1), w2=asf(g2_w2), b2=asf(g2_b2))
        m["ident"] = ident
        in_maps.append(m)
    return in_maps


def kernel(x1, x2, Wkv1, Wkv2, g1_w1, g1_b1, g1_w2, g1_b2,
           g2_w1, g2_b1, g2_w2, g2_b2, _runner=None):
    """Full-input entry point.  Returns (o1, o2), each [4, 4096, 1024] f32."""
    from concourse.bass_utils import run_bass_kernel_spmd

    args = [np.asarray(a, dtype=np.float32) for a in
            (x1, x2, Wkv1, Wkv2, g1_w1, g1_b1, g1_w2, g1_b2,
             g2_w1, g2_b1, g2_w2, g2_b2)]
    with_bias = bool(np.any(args[7]) or np.any(args[11]))  # g1_b2, g2_b2
    nc = _get_program(with_bias)
    in_maps = make_in_maps(*args)
    if _runner is None:
        res = run_bass_kernel_spmd(nc, in_maps, core_ids=list(range(8)))
        results = res.results
    else:
        results = _runner(nc, in_maps)

    B = x1.shape[0]
    o1 = np.empty((B, N, C), dtype=np.float32)
    o2 = np.empty((B, N, C), dtype=np.float32)
    for core in range(8):
        s, b = core // 4, core % 4
        out = results[core]["o"]
        if s == 0:
            o2[b] = out   # core projected x1 -> ctx1 -> o2 = q2 @ ctx1
        else:
            o1[b] = out
    return (o1, o2)
